# revision 44
# baseline (speedup 1.0000x reference)
# Multi-head attention (N=2, S=2048, E=2048, H=16, Dk=128) on 8 NeuronCores.
#
# Sharding: 2 batches x 16 heads = 32 (n,h) pairs -> core c owns batch c//4,
# heads (c%4)*4 .. +4. The reference reshapes (N,H,S,Dk)->(N,S,H*Dk) without
# a head transpose, so rows [h*128,(h+1)*128) of the pre-projection matrix X
# (and hence of the final output) depend on head h only: each core computes
# 512 disjoint output rows and the host concatenates. No collectives.
#
# v6: all matmul operands fp16 (same PE rate as bf16/fp32r at N=512 but 4x
# lower quantization error than bf16; FWL weight loads; half DMA/SBUF vs
# fp32). Attention output stays SBUF-resident; Wo is streamed per output
# chunk during a dense O-projection tail. The softmax denominator is an
# in-place DVE add-tree over the expT tiles plus a single ones-matmul column
# sum + fast reciprocal + GpSimd partition-broadcast. Scores and attn@V
# matmuls are interleaved pairwise so the 2-deep score PSUM ring paces PE to
# the Scalar exp stream without stalling; tails run two steps delayed.
import numpy as np

D_MODEL = 2048
NHEAD = 16
DK = 128
N_BATCH = 2
SEQ = 2048
N_CORES = 8
HEADS_PER_CORE = 4


class Cfg:
    def __init__(self, S=SEQ, E=D_MODEL, NH=HEADS_PER_CORE, CH=512):
        assert S % 128 == 0 and E % 128 == 0
        self.S = S          # sequence length
        self.E = E          # model dim (contraction for projections)
        self.NH = NH        # heads per core
        self.CH = CH        # s-chunk width for attention phase
        self.NK = E // 128  # contraction tiles for projections / O-proj
        self.NK2 = self.NK // 2  # k-pair tiles (DMA granularity)
        self.NT = S // 128  # t tiles (attention contraction)
        self.HDc = NH * DK  # head dims per core
        self.RPH = (S * DK) // E  # output rows per head (=128 at full size)
        assert self.RPH == 128, "O-proj layout assumes 128 rows per head"
        self.NCH = S // CH  # number of s-chunks
        assert S % CH == 0 and CH == 512
        self.PCH = 512      # projection / O-proj free-dim chunk
        self.NPC = S // self.PCH   # projection s-chunks
        self.NOC = E // self.PCH   # O-proj output chunks


def build_program(cfg: Cfg):
    import concourse.bass as bass
    import concourse.tile as tile
    from concourse import bacc, mybir
    from contextlib import ExitStack

    fp32 = mybir.dt.float32
    fp16 = mybir.dt.float16
    AF = mybir.ActivationFunctionType

    S, E, NH, CH = cfg.S, cfg.E, cfg.NH, cfg.CH
    NK, NK2, NT, HDc = cfg.NK, cfg.NK2, cfg.NT, cfg.HDc
    PCH, NPC, NOC, NCH = cfg.PCH, cfg.NPC, cfg.NOC, cfg.NCH
    inv_sqrt_dk = 1.0 / float(np.sqrt(DK))

    nc = bacc.Bacc("TRN2", target_bir_lowering=False, debug=False,
                   num_devices=N_CORES)

    # DRAM I/O (per-core values supplied via in_maps). x inputs are
    # host-pretiled fp16: [k_pair, s_chunk, partition, 2*512] so every DMA
    # reads contiguous 2KB partition rows.
    xq = nc.dram_tensor("xq", [NK2, NPC, 128, 1024], fp16,
                        kind="ExternalInput").ap()
    xk = nc.dram_tensor("xk", [NK2, NPC, 128, 1024], fp16,
                        kind="ExternalInput").ap()
    xv = nc.dram_tensor("xv", [NK2, NPC, 128, 1024], fp16,
                        kind="ExternalInput").ap()
    wq = nc.dram_tensor("wq", [NK2, 128, 2, HDc], fp16,
                        kind="ExternalInput").ap()
    wk = nc.dram_tensor("wk", [NK2, 128, 2, HDc], fp16,
                        kind="ExternalInput").ap()
    wv = nc.dram_tensor("wv", [NK2, 128, 2, HDc], fp16,
                        kind="ExternalInput").ap()
    wo = nc.dram_tensor("wo", [NOC, NK2, 128, 2, 512], fp16,
                        kind="ExternalInput").ap()
    bq = nc.dram_tensor("bq", [128, NH], fp32, kind="ExternalInput").ap()
    bk = nc.dram_tensor("bk", [128, NH], fp32, kind="ExternalInput").ap()
    bv = nc.dram_tensor("bv", [1, HDc], fp16, kind="ExternalInput").ap()
    bo = nc.dram_tensor("bo", [1, E], fp16, kind="ExternalInput").ap()
    ones_d = nc.dram_tensor("ones", [128, 128], fp16,
                            kind="ExternalInput").ap()
    out = nc.dram_tensor("out", [NH * 128, E], fp32, kind="ExternalOutput").ap()

    with tile.TileContext(nc) as tc, ExitStack() as ctx:
        consts = ctx.enter_context(tc.tile_pool(name="consts", bufs=1))
        ones_sb = consts.tile([128, 128], fp16)
        ones_col = ones_sb[:, :1]
        bq_sb = consts.tile([128, NH], fp32)
        bk_sb = consts.tile([128, NH], fp32)
        bv_sb = consts.tile([1, HDc], fp16)
        bo_sb = consts.tile([1, E], fp16)
        bv_bc = consts.tile([128, HDc], fp16)   # bv broadcast along t

        persist = ctx.enter_context(tc.tile_pool(name="persist", bufs=1))
        qc = persist.tile([128, NH, S], fp16)      # qT_c: [d, h, s]
        kc = persist.tile([128, NH, S], fp16)      # kT_c: [d, h, s]
        vc = persist.tile([128, NT, HDc], fp16)    # v_c: [t_p, t_t, h*128+d]
        oc = persist.tile([128, NH, S], fp16)      # attn out: [d, h, s]

        # SBUF pools that span phase A tail + phase B (pre-scored pairs).
        expp = ctx.enter_context(tc.tile_pool(name="expp", bufs=3))
        accp = ctx.enter_context(tc.tile_pool(name="accp", bufs=2))
        bsc = ctx.enter_context(tc.tile_pool(name="bsc", bufs=2))
        expT_of = {}

        # ============== Phase A: q/k/v projections ==============
        from contextlib import ExitStack as _ES
        a_ctx = _ES()
        wpool = a_ctx.enter_context(tc.tile_pool(name="wpool", bufs=2))
        xin = a_ctx.enter_context(tc.tile_pool(name="xin", bufs=6))
        with tc.tile_pool(name="pa_psum", bufs=2, space="PSUM") as pa:

            def load_w(w_dram):
                # 4 sub-tiles of 2 k-pairs each so the first matmuls only
                # wait on the first pair of weight DMAs, not all eight.
                parts = []
                for i in range(4):
                    p = wpool.tile([128, 4, HDc], fp16, tag=f"w{i}",
                                   name=f"w{i}")
                    for j in range(2):
                        nc.scalar.dma_start(p[:, 2 * j:2 * j + 2, :],
                                            w_dram[2 * i + j])
                    parts.append(p)
                return parts

            def proj_qk(w_sb, x_dram, bias_sb, dst, eng):
                # dst[:, m, s*] = W_c @ x^T  (hd x S), bias fused in evict
                for s in range(NPC):
                    ps = [pa.tile([128, PCH], fp32, tag=f"pa{m}",
                                  name=f"pa{m}") for m in range(NH)]
                    for k2 in range(NK2):
                        xtile = xin.tile([128, 1024], fp16, tag="xin")
                        eng.dma_start(xtile[:], x_dram[k2, s])
                        for kk in range(2):
                            k = 2 * k2 + kk
                            xs = xtile[:, kk * 512:(kk + 1) * 512]
                            for m in range(NH):
                                nc.tensor.matmul(
                                    ps[m][:],
                                    w_sb[k // 4][:, k % 4,
                                                 m * 128:(m + 1) * 128],
                                    xs, start=(k == 0), stop=(k == NK - 1))
                    for m in range(NH):
                        nc.scalar.activation(
                            dst[:, m, s * PCH:(s + 1) * PCH],
                            ps[m][:], AF.Identity, bias=bias_sb[:, m:m + 1])

            wq_sb = load_w(wq)
            nc.scalar.dma_start(ones_sb[:], ones_d)
            nc.scalar.dma_start(bq_sb[:], bq)
            nc.scalar.dma_start(bk_sb[:], bk)
            nc.scalar.dma_start(bv_sb[:], bv)
            nc.scalar.dma_start(bo_sb[:], bo)
            proj_qk(wq_sb, xq, bq_sb, qc, nc.sync)
            wk_sb = load_w(wk)
            proj_qk(wk_sb, xk, bk_sb, kc, nc.gpsimd)
            wv_sb = load_w(wv)

            # bv broadcast tile (GpSimd partition-broadcast)
            nc.gpsimd.partition_broadcast(bv_bc[:], bv_sb[:])

        # pa closed: scores PSUM ring opens (spans pre-scores + phase B)
        stp = ctx.enter_context(
            tc.tile_pool(name="st_psum", bufs=2, space="PSUM"))

        def emit_scores(cur):
            # scores + exp for one (h, c) pair; expT kept for the attnV
            # step that consumes it later.
            h, c = cur
            cs = slice(c * CH, (c + 1) * CH)
            expT = expp.tile([128, NT, CH], fp16, tag="expT",
                             name=f"expT_{h}_{c}")
            for tt2 in range(NT // 2):
                st = stp.tile([128, 2, 512], fp32, tag="st", name="st")
                for i in range(2):
                    tt = 2 * tt2 + i
                    nc.tensor.matmul(
                        st[:, i, :], kc[:, h, tt * 128:(tt + 1) * 128],
                        qc[:, h, cs], start=True, stop=True)
                nc.scalar.activation(expT[:, 2 * tt2:2 * tt2 + 2, :],
                                     st[:], AF.Exp, scale=inv_sqrt_dk)
            expT_of[cur] = expT

        atts = [(h, c) for h in range(NH) for c in range(NCH)]
        # Pre-score two pairs so Scalar banks a 2-pair exp lead during the
        # v-projection (it is idle there); the lead absorbs Scalar's
        # per-step deficit for the whole attention phase.
        emit_scores(atts[0])
        emit_scores(atts[1])

        with tc.tile_pool(name="pav_psum", bufs=1, space="PSUM") as pav:

            def proj_v_group(tc4):
                # 4 t-tiles of v: stationary = x tile slices, moving = w
                ps = [pav.tile([128, HDc], fp32, tag=f"pav{j}",
                               name=f"pav{j}") for j in range(4)]
                for k2 in range(NK2):
                    xtile = xin.tile([128, 1024], fp16, tag="xin")
                    eng = nc.sync if k2 % 2 == 0 else nc.gpsimd
                    eng.dma_start(xtile[:], xv[k2, tc4])
                    for kk in range(2):
                        k = 2 * k2 + kk
                        for j in range(4):
                            xs = xtile[:, kk * 512 + j * 128:
                                       kk * 512 + (j + 1) * 128]
                            nc.tensor.matmul(
                                ps[j][:], xs, wv_sb[k // 4][:, k % 4, :],
                                start=(k == 0), stop=(k == NK - 1))
                for j in range(4):
                    nc.vector.tensor_add(vc[:, tc4 * 4 + j, :], ps[j][:],
                                         bv_bc[:])

            for g in range(NT // 4):
                proj_v_group(g)

        a_ctx.close()

        # ============== Phase B: attention ==============
        wop = ctx.enter_context(tc.tile_pool(name="wop", bufs=4))
        wo_tiles = {}

        def load_wo(nn):
            # tiny vc-sourced copies gate each DMA so the scheduler cannot
            # hoist these dep-free loads into the phase-A DMA window; the
            # DMA issues go on sync (idle in early B) so they never delay
            # the per-tail partition-broadcasts on gpsimd.
            wo_t = wop.tile([128, NK, 512], fp16, tag="wo", name=f"wo{nn}")
            for k2 in range(NK2):
                nc.gpsimd.tensor_copy(wo_t[:, 2 * k2, 0:1],
                                      vc[:, NT - 1, HDc - 1:HDc])
                nc.sync.dma_start(wo_t[:, 2 * k2:2 * k2 + 2, :],
                                  wo[nn, k2])
            wo_tiles[nn] = wo_t

        with tc.tile_pool(name="op_psum", bufs=2, space="PSUM") as opp, \
             tc.tile_pool(name="pso_psum", bufs=1, space="PSUM") as psop, \
             tc.tile_pool(name="dn_psum", bufs=1, space="PSUM") as dnp:

            def emit_tree(pair, op):
                # denominator part 1: in-place DVE add-tree over the expT
                # tiles (runs after the attnV matmuls have consumed them).
                pexp = expT_of.pop(pair)
                nc.vector.tensor_add(pexp[:, 0:8, :], pexp[:, 0:8, :],
                                     pexp[:, 8:16, :])
                nc.vector.tensor_add(pexp[:, 0:4, :], pexp[:, 0:4, :],
                                     pexp[:, 4:8, :])
                nc.vector.tensor_add(pexp[:, 0:2, :], pexp[:, 0:2, :],
                                     pexp[:, 2:4, :])
                acc = accp.tile([128, CH], fp16, tag="acc", name="acc")
                nc.vector.tensor_add(acc[:], pexp[:, 0, :], pexp[:, 1, :])
                return (pair, acc, op)

            def emit_tail(t):
                # denominator part 2: ones-matmul column sum, fast
                # reciprocal, GpSimd partition-broadcast, normalize-evict.
                (ph, pc), acc, op = t
                pcs = slice(pc * CH, (pc + 1) * CH)
                dn = dnp.tile([1, CH], fp32, tag="dn", name="dn")
                nc.tensor.matmul(dn[:], ones_col, acc[:],
                                 start=True, stop=True)
                rsc1 = bsc.tile([1, CH], fp32, tag="rsc1", name="rsc1")
                nc.vector.reciprocal_approx_fast(rsc1[:], dn[:])
                rsc = bsc.tile([128, CH], fp32, tag="rsc", name="rsc")
                nc.gpsimd.partition_broadcast(rsc[:], rsc1[:])
                nc.vector.tensor_mul(oc[:, ph, pcs], op[:], rsc[:])

            osb = ctx.enter_context(tc.tile_pool(name="osb", bufs=2))
            obc = ctx.enter_context(tc.tile_pool(name="obc", bufs=1))
            bo_bc = obc.tile([128, E], fp16, name="bo_bc")
            nc.gpsimd.partition_broadcast(bo_bc[:], bo_sb[:])

            def start_chunk(h, nn, pool=None, tag="pso"):
                pool = pool or psop
                return {"h": h, "nn": nn, "k": 0,
                        "ps": pool.tile([128, PCH], fp32, tag=tag,
                                        name="pso"),
                        "ocv": oc[:, h, :].rearrange("p (j i) -> p i j",
                                                     i=NK),
                        "wo": wo_tiles[nn]}

            def chunk_mms(ch, n):
                for _ in range(n):
                    if ch is None or ch["k"] >= NK:
                        return
                    k = ch["k"]
                    nc.tensor.matmul(ch["ps"][:], ch["ocv"][:, k, :],
                                     ch["wo"][:, k, :],
                                     start=(k == 0), stop=(k == NK - 1))
                    ch["k"] += 1

            def finish_chunk(ch):
                if ch is None:
                    return
                chunk_mms(ch, NK - ch["k"])
                ns = slice(ch["nn"] * PCH, (ch["nn"] + 1) * PCH)
                ot = osb.tile([128, PCH], fp32, tag="osb")
                nc.vector.tensor_add(ot[:], ch["ps"][:], bo_bc[:, ns])
                nc.sync.dma_start(
                    out[ch["h"] * 128:(ch["h"] + 1) * 128, ns], ot[:])

            def att_step(av_pair, sc_pair, pending, ch):
                # Interleave scores(j+2) tt-pairs with attnV(j) tt-pairs
                # plus two O-proj matmuls per slot: the filler keeps each
                # st-ring slot interval above the Scalar exp latency so the
                # score stream is never exp-paced.
                ah, ac = av_pair
                aexp = expT_of[av_pair]
                op = opp.tile([128, CH], fp32, tag="op", name="op")
                expT = None
                if sc_pair is not None:
                    h, c = sc_pair
                    cs = slice(c * CH, (c + 1) * CH)
                    expT = expp.tile([128, NT, CH], fp16, tag="expT",
                                     name=f"expT_{h}_{c}")
                for tt2 in range(NT // 2):
                    if sc_pair is not None:
                        st = stp.tile([128, 2, 512], fp32, tag="st",
                                      name="st")
                        for i in range(2):
                            tt = 2 * tt2 + i
                            nc.tensor.matmul(
                                st[:, i, :],
                                kc[:, h, tt * 128:(tt + 1) * 128],
                                qc[:, h, cs], start=True, stop=True)
                        nc.scalar.activation(expT[:, 2 * tt2:2 * tt2 + 2, :],
                                             st[:], AF.Exp,
                                             scale=inv_sqrt_dk)
                    for i in range(2):
                        tt = 2 * tt2 + i
                        nc.tensor.matmul(
                            op[:], vc[:, tt, ah * 128:(ah + 1) * 128],
                            aexp[:, tt, :], start=(tt == 0),
                            stop=(tt == NT - 1))
                    chunk_mms(ch, 2)
                    if tt2 == 2 and pending:
                        emit_tail(pending.pop(0))
                finish_chunk(ch)
                if sc_pair is not None:
                    expT_of[sc_pair] = expT
                pending.append(emit_tree(av_pair, op))

            chunk_queue = [(h, nn) for h in range(NH) for nn in range(NOC)]
            pending = []
            for j, cur in enumerate(atts):
                sc_pair = atts[j + 2] if j + 2 < len(atts) else None
                ch = None
                if chunk_queue and j >= 4 * (chunk_queue[0][0] + 1) + 1:
                    ch = start_chunk(*chunk_queue.pop(0))
                att_step(cur, sc_pair, pending, ch)
                # Wo chunk slices stream in during the first attention
                # steps (x traffic is over; gpsimd queue is idle)
                if j < NOC:
                    load_wo(j)

            # ===== drain: last tail + remaining O-proj chunks =====
            # alternate the PSUM ring between the pso and (now idle) op
            # tags so back-to-back chunks never wait on their own evict
            ch = start_chunk(*chunk_queue.pop(0), pool=opp, tag="op")
            chunk_mms(ch, NK)
            finish_chunk(ch)
            emit_tail(pending.pop(0))
            alt = 0
            while chunk_queue:
                if alt % 2 == 0:
                    ch = start_chunk(*chunk_queue.pop(0))
                else:
                    ch = start_chunk(*chunk_queue.pop(0), pool=opp, tag="op")
                alt += 1
                chunk_mms(ch, NK)
                finish_chunk(ch)

    nc.compile()
    return nc


def _tile_x(xt, NK2, NPC):
    # (E, S) fp16 -> [k_pair, s_chunk, 128, 2*512] contiguous (2KB rows)
    return np.ascontiguousarray(
        xt.reshape(NK2, 2, 128, NPC, 512).transpose(0, 3, 2, 1, 4)
        .reshape(NK2, NPC, 128, 1024))


def _tile_w(wT, NK2, HDc):
    # (E, HDc) fp16 -> [k_pair, 128, 2, HDc] (2KB rows)
    return np.ascontiguousarray(
        wT.reshape(NK2, 2, 128, HDc).transpose(0, 2, 1, 3))


def shard_inputs(cfg: Cfg, query, key, value, Wq, bq, Wk, bk, Wv, bv, Wo, bo):
    """Build per-core in_maps from full inputs."""
    f = np.float32
    h16 = np.float16
    query, key, value = (np.asarray(a, f) for a in (query, key, value))
    Wq, Wk, Wv, Wo = (np.asarray(a, f) for a in (Wq, Wk, Wv, Wo))
    bq, bk, bv, bo = (np.asarray(a, f) for a in (bq, bk, bv, bo))
    NH, HDc, NK2, NPC = cfg.NH, cfg.HDc, cfg.NK2, cfg.NPC
    NOC = cfg.NOC
    # Wo^T -> [nn, k_pair, 128, 2, 512] (2KB rows)
    wo_t = np.ascontiguousarray(
        Wo.T.astype(h16).reshape(NK2, 2, 128, NOC, 512)
        .transpose(3, 0, 2, 1, 4))
    _ONES = np.ones((128, 128), np.float32)
    bo_r = np.ascontiguousarray(bo.reshape(1, -1))
    xq_t = [_tile_x(query[n].T.astype(h16), NK2, NPC) for n in range(N_BATCH)]
    xk_t = [_tile_x(key[n].T.astype(h16), NK2, NPC) for n in range(N_BATCH)]
    xv_t = [_tile_x(value[n].T.astype(h16), NK2, NPC) for n in range(N_BATCH)]
    in_maps = []
    cores_per_batch = N_CORES // N_BATCH
    for c in range(N_CORES):
        n = c // cores_per_batch
        hs = (c % cores_per_batch) * HDc
        sl = slice(hs, hs + HDc)
        in_maps.append({
            "xq": xq_t[n],
            "xk": xk_t[n],
            "xv": xv_t[n],
            "wq": _tile_w(np.ascontiguousarray(Wq[sl].T).astype(h16),
                          NK2, HDc),
            "wk": _tile_w(np.ascontiguousarray(Wk[sl].T).astype(h16),
                          NK2, HDc),
            "wv": _tile_w(np.ascontiguousarray(Wv[sl].T).astype(h16),
                          NK2, HDc),
            "wo": wo_t,
            "bq": np.ascontiguousarray(bq[sl].reshape(NH, 128).T),
            "bk": np.ascontiguousarray(bk[sl].reshape(NH, 128).T),
            "bv": np.ascontiguousarray(bv[sl].reshape(1, HDc)).astype(h16),
            "bo": bo_r.astype(h16),
            "ones": _ONES.astype(h16),
        })
    return in_maps


def gather_outputs(cfg: Cfg, results):
    """results: list of per-core {'out': (NH*128, E)} -> full (N, S, E)."""
    E = cfg.E
    full = np.empty((N_BATCH, SEQ, E), np.float32)
    cores_per_batch = N_CORES // N_BATCH
    rows = cfg.NH * 128
    for c in range(N_CORES):
        n = c // cores_per_batch
        r0 = (c % cores_per_batch) * rows
        full[n, r0:r0 + rows, :] = results[c]["out"]
    return full


_CACHE = {}


def kernel(**inputs) -> np.ndarray:
    from concourse.bass_utils import run_bass_kernel_spmd
    cfg = Cfg()
    if "nc" not in _CACHE:
        _CACHE["nc"] = build_program(cfg)
    nc = _CACHE["nc"]
    in_maps = shard_inputs(cfg, **inputs)
    res = run_bass_kernel_spmd(nc, in_maps, core_ids=list(range(N_CORES)))
    return gather_outputs(cfg, res.results)


# revision 46
# speedup vs baseline: 1.0083x; 1.0083x over previous
# Multi-head attention (N=2, S=2048, E=2048, H=16, Dk=128) on 8 NeuronCores.
#
# Sharding: 2 batches x 16 heads = 32 (n,h) pairs -> core c owns batch c//4,
# heads (c%4)*4 .. +4. The reference reshapes (N,H,S,Dk)->(N,S,H*Dk) without
# a head transpose, so rows [h*128,(h+1)*128) of the pre-projection matrix X
# (and hence of the final output) depend on head h only: each core computes
# 512 disjoint output rows and the host concatenates. No collectives.
#
# v6: all matmul operands fp16 (same PE rate as bf16/fp32r at N=512 but 4x
# lower quantization error than bf16; FWL weight loads; half DMA/SBUF vs
# fp32). Attention output stays SBUF-resident; Wo is streamed per output
# chunk during a dense O-projection tail. The softmax denominator is an
# in-place DVE add-tree over the expT tiles plus a single ones-matmul column
# sum + fast reciprocal + GpSimd partition-broadcast. Scores and attn@V
# matmuls are interleaved pairwise so the 2-deep score PSUM ring paces PE to
# the Scalar exp stream without stalling; tails run two steps delayed.
import numpy as np

D_MODEL = 2048
NHEAD = 16
DK = 128
N_BATCH = 2
SEQ = 2048
N_CORES = 8
HEADS_PER_CORE = 4


class Cfg:
    def __init__(self, S=SEQ, E=D_MODEL, NH=HEADS_PER_CORE, CH=512):
        assert S % 128 == 0 and E % 128 == 0
        self.S = S          # sequence length
        self.E = E          # model dim (contraction for projections)
        self.NH = NH        # heads per core
        self.CH = CH        # s-chunk width for attention phase
        self.NK = E // 128  # contraction tiles for projections / O-proj
        self.NK2 = self.NK // 2  # k-pair tiles (DMA granularity)
        self.NT = S // 128  # t tiles (attention contraction)
        self.HDc = NH * DK  # head dims per core
        self.RPH = (S * DK) // E  # output rows per head (=128 at full size)
        assert self.RPH == 128, "O-proj layout assumes 128 rows per head"
        self.NCH = S // CH  # number of s-chunks
        assert S % CH == 0 and CH == 512
        self.PCH = 512      # projection / O-proj free-dim chunk
        self.NPC = S // self.PCH   # projection s-chunks
        self.NOC = E // self.PCH   # O-proj output chunks


def build_program(cfg: Cfg):
    import concourse.bass as bass
    import concourse.tile as tile
    from concourse import bacc, mybir
    from contextlib import ExitStack

    fp32 = mybir.dt.float32
    fp16 = mybir.dt.float16
    AF = mybir.ActivationFunctionType

    S, E, NH, CH = cfg.S, cfg.E, cfg.NH, cfg.CH
    NK, NK2, NT, HDc = cfg.NK, cfg.NK2, cfg.NT, cfg.HDc
    PCH, NPC, NOC, NCH = cfg.PCH, cfg.NPC, cfg.NOC, cfg.NCH
    inv_sqrt_dk = 1.0 / float(np.sqrt(DK))

    nc = bacc.Bacc("TRN2", target_bir_lowering=False, debug=False,
                   num_devices=N_CORES)

    # DRAM I/O (per-core values supplied via in_maps). x inputs are
    # host-pretiled fp16: [k_pair, s_chunk, partition, 2*512] so every DMA
    # reads contiguous 2KB partition rows.
    xq = nc.dram_tensor("xq", [NK2, NPC, 128, 1024], fp16,
                        kind="ExternalInput").ap()
    xk = nc.dram_tensor("xk", [NK2, NPC, 128, 1024], fp16,
                        kind="ExternalInput").ap()
    xv = nc.dram_tensor("xv", [NK2, NPC, 128, 1024], fp16,
                        kind="ExternalInput").ap()
    wq = nc.dram_tensor("wq", [NK2, 128, 2, HDc], fp16,
                        kind="ExternalInput").ap()
    wk = nc.dram_tensor("wk", [NK2, 128, 2, HDc], fp16,
                        kind="ExternalInput").ap()
    wv = nc.dram_tensor("wv", [NK2, 128, 2, HDc], fp16,
                        kind="ExternalInput").ap()
    wo = nc.dram_tensor("wo", [NOC, NK2, 128, 2, 512], fp16,
                        kind="ExternalInput").ap()
    bq = nc.dram_tensor("bq", [128, NH], fp32, kind="ExternalInput").ap()
    bk = nc.dram_tensor("bk", [128, NH], fp32, kind="ExternalInput").ap()
    bv = nc.dram_tensor("bv", [1, HDc], fp16, kind="ExternalInput").ap()
    bo = nc.dram_tensor("bo", [1, E], fp16, kind="ExternalInput").ap()
    ones_d = nc.dram_tensor("ones", [128, 128], fp16,
                            kind="ExternalInput").ap()
    out = nc.dram_tensor("out", [NH * 128, E], fp32, kind="ExternalOutput").ap()

    with tile.TileContext(nc) as tc, ExitStack() as ctx:
        consts = ctx.enter_context(tc.tile_pool(name="consts", bufs=1))
        ones_sb = consts.tile([128, 128], fp16)
        ones_col = ones_sb[:, :1]
        bq_sb = consts.tile([128, NH], fp32)
        bk_sb = consts.tile([128, NH], fp32)
        bv_sb = consts.tile([1, HDc], fp16)
        bo_sb = consts.tile([1, E], fp16)
        bv_bc = consts.tile([128, HDc], fp16)   # bv broadcast along t

        persist = ctx.enter_context(tc.tile_pool(name="persist", bufs=1))
        qc = persist.tile([128, NH, S], fp16)      # qT_c: [d, h, s]
        kc = persist.tile([128, NH, S], fp16)      # kT_c: [d, h, s]
        vc = persist.tile([128, NT, HDc], fp16)    # v_c: [t_p, t_t, h*128+d]
        oc = persist.tile([128, NH, S], fp16)      # attn out: [d, h, s]

        # SBUF pools that span phase A tail + phase B (pre-scored pairs).
        expp = ctx.enter_context(tc.tile_pool(name="expp", bufs=3))
        accp = ctx.enter_context(tc.tile_pool(name="accp", bufs=2))
        bsc = ctx.enter_context(tc.tile_pool(name="bsc", bufs=2))
        expT_of = {}

        # ============== Phase A: q/k/v projections ==============
        from contextlib import ExitStack as _ES
        a_ctx = _ES()
        wpool = a_ctx.enter_context(tc.tile_pool(name="wpool", bufs=2))
        xin = a_ctx.enter_context(tc.tile_pool(name="xin", bufs=6))
        with tc.tile_pool(name="pa_psum", bufs=2, space="PSUM") as pa:

            def load_w(w_dram):
                # 4 sub-tiles of 2 k-pairs each so the first matmuls only
                # wait on the first pair of weight DMAs, not all eight.
                parts = []
                for i in range(4):
                    p = wpool.tile([128, 4, HDc], fp16, tag=f"w{i}",
                                   name=f"w{i}")
                    for j in range(2):
                        nc.scalar.dma_start(p[:, 2 * j:2 * j + 2, :],
                                            w_dram[2 * i + j])
                    parts.append(p)
                return parts

            def proj_qk(w_sb, x_dram, bias_sb, dst, eng):
                # dst[:, m, s*] = W_c @ x^T  (hd x S), bias fused in evict
                for s in range(NPC):
                    ps = [pa.tile([128, PCH], fp32, tag=f"pa{m}",
                                  name=f"pa{m}") for m in range(NH)]
                    for k2 in range(NK2):
                        xtile = xin.tile([128, 1024], fp16, tag="xin")
                        eng.dma_start(xtile[:], x_dram[k2, s])
                        for kk in range(2):
                            k = 2 * k2 + kk
                            xs = xtile[:, kk * 512:(kk + 1) * 512]
                            for m in range(NH):
                                nc.tensor.matmul(
                                    ps[m][:],
                                    w_sb[k // 4][:, k % 4,
                                                 m * 128:(m + 1) * 128],
                                    xs, start=(k == 0), stop=(k == NK - 1))
                    for m in range(NH):
                        nc.scalar.activation(
                            dst[:, m, s * PCH:(s + 1) * PCH],
                            ps[m][:], AF.Identity, bias=bias_sb[:, m:m + 1])

            wq_sb = load_w(wq)
            nc.scalar.dma_start(ones_sb[:], ones_d)
            nc.scalar.dma_start(bq_sb[:], bq)
            nc.scalar.dma_start(bk_sb[:], bk)
            nc.scalar.dma_start(bv_sb[:], bv)
            nc.scalar.dma_start(bo_sb[:], bo)
            proj_qk(wq_sb, xq, bq_sb, qc, nc.sync)
            wk_sb = load_w(wk)
            proj_qk(wk_sb, xk, bk_sb, kc, nc.gpsimd)
            wv_sb = load_w(wv)

            # bv broadcast tile (GpSimd partition-broadcast)
            nc.gpsimd.partition_broadcast(bv_bc[:], bv_sb[:])

        # pa closed: scores PSUM ring opens (spans pre-scores + phase B)
        stp = ctx.enter_context(
            tc.tile_pool(name="st_psum", bufs=2, space="PSUM"))

        def emit_scores(cur):
            # scores + exp for one (h, c) pair; expT kept for the attnV
            # step that consumes it later.
            h, c = cur
            cs = slice(c * CH, (c + 1) * CH)
            expT = expp.tile([128, NT, CH], fp16, tag="expT",
                             name=f"expT_{h}_{c}")
            for tt2 in range(NT // 2):
                st = stp.tile([128, 2, 512], fp32, tag="st", name="st")
                for i in range(2):
                    tt = 2 * tt2 + i
                    nc.tensor.matmul(
                        st[:, i, :], kc[:, h, tt * 128:(tt + 1) * 128],
                        qc[:, h, cs], start=True, stop=True)
                nc.scalar.activation(expT[:, 2 * tt2:2 * tt2 + 2, :],
                                     st[:], AF.Exp, scale=inv_sqrt_dk)
            expT_of[cur] = expT

        atts = [(h, c) for h in range(NH) for c in range(NCH)]
        # Pre-score two pairs so Scalar banks a 2-pair exp lead during the
        # v-projection (it is idle there); the lead absorbs Scalar's
        # per-step deficit for the whole attention phase.
        emit_scores(atts[0])
        emit_scores(atts[1])

        with tc.tile_pool(name="pav_psum", bufs=1, space="PSUM") as pav:

            def proj_v_group(tc4):
                # 4 t-tiles of v: stationary = x tile slices, moving = w
                ps = [pav.tile([128, HDc], fp32, tag=f"pav{j}",
                               name=f"pav{j}") for j in range(4)]
                for k2 in range(NK2):
                    xtile = xin.tile([128, 1024], fp16, tag="xin")
                    eng = nc.sync if k2 % 2 == 0 else nc.gpsimd
                    eng.dma_start(xtile[:], xv[k2, tc4])
                    for kk in range(2):
                        k = 2 * k2 + kk
                        for j in range(4):
                            xs = xtile[:, kk * 512 + j * 128:
                                       kk * 512 + (j + 1) * 128]
                            nc.tensor.matmul(
                                ps[j][:], xs, wv_sb[k // 4][:, k % 4, :],
                                start=(k == 0), stop=(k == NK - 1))
                for j in range(4):
                    nc.vector.tensor_add(vc[:, tc4 * 4 + j, :], ps[j][:],
                                         bv_bc[:])

            for g in range(NT // 4):
                proj_v_group(g)

        a_ctx.close()

        # ============== Phase B: attention ==============
        wop = ctx.enter_context(tc.tile_pool(name="wop", bufs=4))
        wo_tiles = {}

        def load_wo(nn):
            # tiny vc-sourced copies gate each DMA so the scheduler cannot
            # hoist these dep-free loads into the phase-A DMA window; the
            # DMA issues go on sync (idle in early B) so they never delay
            # the per-tail partition-broadcasts on gpsimd.
            wo_t = wop.tile([128, NK, 512], fp16, tag="wo", name=f"wo{nn}")
            for k2 in range(NK2):
                nc.gpsimd.tensor_copy(wo_t[:, 2 * k2, 0:1],
                                      vc[:, NT - 1, HDc - 1:HDc])
                nc.sync.dma_start(wo_t[:, 2 * k2:2 * k2 + 2, :],
                                  wo[nn, k2])
            wo_tiles[nn] = wo_t

        with tc.tile_pool(name="op_psum", bufs=2, space="PSUM") as opp, \
             tc.tile_pool(name="pso_psum", bufs=1, space="PSUM") as psop, \
             tc.tile_pool(name="dn_psum", bufs=1, space="PSUM") as dnp:

            def emit_tree(pair, op):
                # denominator part 1: in-place DVE add-tree over the expT
                # tiles (runs after the attnV matmuls have consumed them).
                pexp = expT_of.pop(pair)
                nc.vector.tensor_add(pexp[:, 0:8, :], pexp[:, 0:8, :],
                                     pexp[:, 8:16, :])
                nc.vector.tensor_add(pexp[:, 0:4, :], pexp[:, 0:4, :],
                                     pexp[:, 4:8, :])
                nc.vector.tensor_add(pexp[:, 0:2, :], pexp[:, 0:2, :],
                                     pexp[:, 2:4, :])
                acc = accp.tile([128, CH], fp16, tag="acc", name="acc")
                nc.vector.tensor_add(acc[:], pexp[:, 0, :], pexp[:, 1, :])
                return (pair, acc, op)

            def emit_tail(t):
                # denominator part 2: ones-matmul column sum, fast
                # reciprocal, GpSimd partition-broadcast, normalize-evict.
                (ph, pc), acc, op = t
                pcs = slice(pc * CH, (pc + 1) * CH)
                dn = dnp.tile([1, CH], fp32, tag="dn", name="dn")
                nc.tensor.matmul(dn[:], ones_col, acc[:],
                                 start=True, stop=True)
                rsc1 = bsc.tile([1, CH], fp32, tag="rsc1", name="rsc1")
                nc.vector.reciprocal_approx_fast(rsc1[:], dn[:])
                rsc = bsc.tile([128, CH], fp32, tag="rsc", name="rsc")
                nc.gpsimd.partition_broadcast(rsc[:], rsc1[:])
                nc.vector.tensor_mul(oc[:, ph, pcs], op[:], rsc[:])

            osb = ctx.enter_context(tc.tile_pool(name="osb", bufs=2))
            obc = ctx.enter_context(tc.tile_pool(name="obc", bufs=1))
            bo_bc = obc.tile([128, E], fp16, name="bo_bc")
            nc.gpsimd.partition_broadcast(bo_bc[:], bo_sb[:])

            def start_chunk(h, nn, pool=None, tag="pso"):
                pool = pool or psop
                return {"h": h, "nn": nn, "k": 0,
                        "ps": pool.tile([128, PCH], fp32, tag=tag,
                                        name="pso"),
                        "ocv": oc[:, h, :].rearrange("p (j i) -> p i j",
                                                     i=NK),
                        "wo": wo_tiles[nn]}

            def chunk_mms(ch, n):
                for _ in range(n):
                    if ch is None or ch["k"] >= NK:
                        return
                    k = ch["k"]
                    nc.tensor.matmul(ch["ps"][:], ch["ocv"][:, k, :],
                                     ch["wo"][:, k, :],
                                     start=(k == 0), stop=(k == NK - 1))
                    ch["k"] += 1

            def finish_chunk(ch):
                if ch is None:
                    return
                chunk_mms(ch, NK - ch["k"])
                ns = slice(ch["nn"] * PCH, (ch["nn"] + 1) * PCH)
                ot = osb.tile([128, PCH], fp32, tag="osb")
                nc.vector.tensor_add(ot[:], ch["ps"][:], bo_bc[:, ns])
                nc.sync.dma_start(
                    out[ch["h"] * 128:(ch["h"] + 1) * 128, ns], ot[:])

            def att_step(av_pair, sc_pair, pending, ch):
                # Interleave scores(j+2) tt-pairs with attnV(j) tt-pairs
                # plus two O-proj matmuls per slot: the filler keeps each
                # st-ring slot interval above the Scalar exp latency so the
                # score stream is never exp-paced.
                ah, ac = av_pair
                aexp = expT_of[av_pair]
                op = opp.tile([128, CH], fp32, tag="op", name="op")
                expT = None
                if sc_pair is not None:
                    h, c = sc_pair
                    cs = slice(c * CH, (c + 1) * CH)
                    expT = expp.tile([128, NT, CH], fp16, tag="expT",
                                     name=f"expT_{h}_{c}")
                for tt2 in range(NT // 2):
                    if sc_pair is not None:
                        st = stp.tile([128, 2, 512], fp32, tag="st",
                                      name="st")
                        for i in range(2):
                            tt = 2 * tt2 + i
                            nc.tensor.matmul(
                                st[:, i, :],
                                kc[:, h, tt * 128:(tt + 1) * 128],
                                qc[:, h, cs], start=True, stop=True)
                        nc.scalar.activation(expT[:, 2 * tt2:2 * tt2 + 2, :],
                                             st[:], AF.Exp,
                                             scale=inv_sqrt_dk)
                    for i in range(2):
                        tt = 2 * tt2 + i
                        nc.tensor.matmul(
                            op[:], vc[:, tt, ah * 128:(ah + 1) * 128],
                            aexp[:, tt, :], start=(tt == 0),
                            stop=(tt == NT - 1))
                    chunk_mms(ch, 2)
                    if tt2 == 2 and pending:
                        emit_tail(pending.pop(0))
                finish_chunk(ch)
                if sc_pair is not None:
                    expT_of[sc_pair] = expT
                pending.append(emit_tree(av_pair, op))

            chunk_queue = [(h, nn) for h in range(NH) for nn in range(NOC)]
            pending = []
            for j, cur in enumerate(atts):
                sc_pair = atts[j + 2] if j + 2 < len(atts) else None
                ch = None
                if chunk_queue and j >= 4 * (chunk_queue[0][0] + 1) + 1:
                    ch = start_chunk(*chunk_queue.pop(0))
                att_step(cur, sc_pair, pending, ch)
                # Wo chunk slices stream in during the first attention
                # steps (x traffic is over; gpsimd queue is idle)
                if j < NOC:
                    load_wo(j)

            # ===== drain: last tail + remaining O-proj chunks =====
            # The spacer chunk is split around the final tail so its
            # matmuls cover the tail's DVE chain (tree -> dn -> recip ->
            # broadcast -> mul); chunks alternate between the pso and op
            # PSUM tags so back-to-back chunks never wait on their evict.
            t = pending.pop(0)
            ch = start_chunk(*chunk_queue.pop(0))
            chunk_mms(ch, 6)
            dn = dnp.tile([1, CH], fp32, tag="dn", name="dn")
            nc.tensor.matmul(dn[:], ones_col, t[1][:], start=True, stop=True)
            rsc1 = bsc.tile([1, CH], fp32, tag="rsc1", name="rsc1")
            nc.vector.reciprocal_approx_fast(rsc1[:], dn[:])
            rsc = bsc.tile([128, CH], fp32, tag="rsc", name="rsc")
            nc.gpsimd.partition_broadcast(rsc[:], rsc1[:])
            chunk_mms(ch, 6)
            (lh, lc) = t[0]
            nc.vector.tensor_mul(oc[:, lh, slice(lc * CH, (lc + 1) * CH)],
                                 t[2][:], rsc[:])
            chunk_mms(ch, NK - ch["k"])
            finish_chunk(ch)
            alt = 0
            while chunk_queue:
                if alt % 2 == 0:
                    ch = start_chunk(*chunk_queue.pop(0), pool=opp, tag="op")
                else:
                    ch = start_chunk(*chunk_queue.pop(0))
                alt += 1
                chunk_mms(ch, NK)
                finish_chunk(ch)

    nc.compile()
    return nc


def _tile_x(xt, NK2, NPC):
    # (E, S) fp16 -> [k_pair, s_chunk, 128, 2*512] contiguous (2KB rows)
    return np.ascontiguousarray(
        xt.reshape(NK2, 2, 128, NPC, 512).transpose(0, 3, 2, 1, 4)
        .reshape(NK2, NPC, 128, 1024))


def _tile_w(wT, NK2, HDc):
    # (E, HDc) fp16 -> [k_pair, 128, 2, HDc] (2KB rows)
    return np.ascontiguousarray(
        wT.reshape(NK2, 2, 128, HDc).transpose(0, 2, 1, 3))


def shard_inputs(cfg: Cfg, query, key, value, Wq, bq, Wk, bk, Wv, bv, Wo, bo):
    """Build per-core in_maps from full inputs."""
    f = np.float32
    h16 = np.float16
    query, key, value = (np.asarray(a, f) for a in (query, key, value))
    Wq, Wk, Wv, Wo = (np.asarray(a, f) for a in (Wq, Wk, Wv, Wo))
    bq, bk, bv, bo = (np.asarray(a, f) for a in (bq, bk, bv, bo))
    NH, HDc, NK2, NPC = cfg.NH, cfg.HDc, cfg.NK2, cfg.NPC
    NOC = cfg.NOC
    # Wo^T -> [nn, k_pair, 128, 2, 512] (2KB rows)
    wo_t = np.ascontiguousarray(
        Wo.T.astype(h16).reshape(NK2, 2, 128, NOC, 512)
        .transpose(3, 0, 2, 1, 4))
    _ONES = np.ones((128, 128), np.float32)
    bo_r = np.ascontiguousarray(bo.reshape(1, -1))
    xq_t = [_tile_x(query[n].T.astype(h16), NK2, NPC) for n in range(N_BATCH)]
    xk_t = [_tile_x(key[n].T.astype(h16), NK2, NPC) for n in range(N_BATCH)]
    xv_t = [_tile_x(value[n].T.astype(h16), NK2, NPC) for n in range(N_BATCH)]
    in_maps = []
    cores_per_batch = N_CORES // N_BATCH
    for c in range(N_CORES):
        n = c // cores_per_batch
        hs = (c % cores_per_batch) * HDc
        sl = slice(hs, hs + HDc)
        in_maps.append({
            "xq": xq_t[n],
            "xk": xk_t[n],
            "xv": xv_t[n],
            "wq": _tile_w(np.ascontiguousarray(Wq[sl].T).astype(h16),
                          NK2, HDc),
            "wk": _tile_w(np.ascontiguousarray(Wk[sl].T).astype(h16),
                          NK2, HDc),
            "wv": _tile_w(np.ascontiguousarray(Wv[sl].T).astype(h16),
                          NK2, HDc),
            "wo": wo_t,
            "bq": np.ascontiguousarray(bq[sl].reshape(NH, 128).T),
            "bk": np.ascontiguousarray(bk[sl].reshape(NH, 128).T),
            "bv": np.ascontiguousarray(bv[sl].reshape(1, HDc)).astype(h16),
            "bo": bo_r.astype(h16),
            "ones": _ONES.astype(h16),
        })
    return in_maps


def gather_outputs(cfg: Cfg, results):
    """results: list of per-core {'out': (NH*128, E)} -> full (N, S, E)."""
    E = cfg.E
    full = np.empty((N_BATCH, SEQ, E), np.float32)
    cores_per_batch = N_CORES // N_BATCH
    rows = cfg.NH * 128
    for c in range(N_CORES):
        n = c // cores_per_batch
        r0 = (c % cores_per_batch) * rows
        full[n, r0:r0 + rows, :] = results[c]["out"]
    return full


_CACHE = {}


def kernel(**inputs) -> np.ndarray:
    from concourse.bass_utils import run_bass_kernel_spmd
    cfg = Cfg()
    if "nc" not in _CACHE:
        _CACHE["nc"] = build_program(cfg)
    nc = _CACHE["nc"]
    in_maps = shard_inputs(cfg, **inputs)
    res = run_bass_kernel_spmd(nc, in_maps, core_ids=list(range(N_CORES)))
    return gather_outputs(cfg, res.results)
